# revision 19
# baseline (speedup 1.0000x reference)
"""Causal self-attention (B=2, T=4096, C=768, H=12, D=64) on 8 Trainium2 cores.

Sharding: (batch, head-group) across 8 cores — core i handles batch i//4,
heads 3*(i%4) .. 3*(i%4)+2.  Each core computes q/k in a transposed [d, T]
layout (S^T formulation: no transposes anywhere in attention), v in natural
[T, d] layout packed next to a ones-block so one AV matmul produces both
y_un^T and the broadcast softmax denominator.  Output projection produces a
partial z[T, C] per core; host sums the 4 partials per batch and adds biases.

Numerics: all matmuls in fp16 (same 10-bit mantissa as TF32/fp32r, but full
PE rate), fp32 PSUM accumulation.  Softmax exp has no max-subtraction; a
constant exp(S-10) shift keeps probs inside fp16 range and cancels in the
normalization.  v-bias and output bias fold into one host-side row:
y @ W_p + b_p == (y0/rowsum) @ W_p + (b_v @ W_p + b_p).

Perf notes (measured on HW): fp16/bf16 matmul N=512 is ~222 ns warm; matmuls
that alternate partition base or contraction row-groups pay ~100 ns per
transition, so all heads keep base-0 operands and S-matmul (K=64) / AV-matmul
(K=128) runs are batched ~4-long.  DVE reciprocal is ~3.4 us per call, so the
normalization chain stages everything to SBUF to free PSUM immediately.
"""
import os
import sys

sys.path.insert(0, "/opt/trn_rl_repo")

import numpy as np

B, T, C = 2, 4096, 768
H, D = 12, 64
HPC = 3            # heads per core
NCORE = 8
QC = 512           # q-chunk (free dim of S^T blocks)
KTS = 128          # k-tile size
NJQ = T // QC      # 8 q-chunks
NKT = T // KTS     # 32 k-tiles
NTT = T // 128     # 32 t-tiles (proj)
NCCH = C // 128    # 6 contraction chunks

# vones column layout: [v0 | ones | v1 | v2 | ones]
VONES_W = 320
V_LHST = [0, 64, 192]    # lhsT col offset per local head ([V|1], [1|V], [V|1])
V_DST = [0, 128, 192]    # where phase A writes each head's v block
EXP_SHIFT = -10.0

_cache = {}
last_results = None  # set by kernel(); test.py reads exec_time_ns off this


def _build():
    import concourse.mybir as mybir
    import concourse.tile as tile
    from concourse import bacc

    F32 = mybir.dt.float32
    F16 = mybir.dt.float16
    AF = mybir.ActivationFunctionType

    nc = bacc.Bacc("TRN2", target_bir_lowering=False, debug=False)

    xT = nc.dram_tensor("xT", [C, T], F16, kind="ExternalInput").ap()
    wqk = nc.dram_tensor("wqk", [C, 384], F16, kind="ExternalInput").ap()
    wv = nc.dram_tensor("wv", [C, 192], F16, kind="ExternalInput").ap()
    wp = nc.dram_tensor("wp", [192, C], F16, kind="ExternalInput").ap()
    bqk = nc.dram_tensor("bqk", [128, 3], F32, kind="ExternalInput").ap()
    trimask = nc.dram_tensor("trimask", [128, 128], F16, kind="ExternalInput").ap()
    z = nc.dram_tensor("z", [T, C], F32, kind="ExternalOutput").ap()
    debug = os.environ.get("CC_ATTN_DEBUG", "0") == "1"
    if debug:
        dq = nc.dram_tensor("dbg_qT", [64, HPC, T], F16, kind="ExternalOutput").ap()
        dk = nc.dram_tensor("dbg_kT", [64, HPC, T], F16, kind="ExternalOutput").ap()
        dv = nc.dram_tensor("dbg_vones", [128, 32 * VONES_W], F16,
                            kind="ExternalOutput").ap()
        dy0 = nc.dram_tensor("dbg_yT0", [128, T], F16, kind="ExternalOutput").ap()
        dy1 = nc.dram_tensor("dbg_yT1", [64, T], F16, kind="ExternalOutput").ap()

    with tile.TileContext(nc) as tc:
        with tc.tile_pool(name="persist", bufs=1) as persist:
            qT = persist.tile([64, HPC, T], F16, tag="qT")
            kT = persist.tile([64, HPC, T], F16, tag="kT")
            vones = persist.tile([128, NKT, VONES_W], F16, tag="vones")
            yT0 = persist.tile([128, T], F16, tag="yT0")
            yT1 = persist.tile([64, T], F16, tag="yT1")
            bqk_sb = persist.tile([128, 3], F32, tag="bqk")
            shift_sb = persist.tile([128, 1], F32, tag="shift")
            tri_sb = persist.tile([128, 128], F16, tag="tri")

            nc.sync.dma_start(bqk_sb[:], bqk)
            nc.sync.dma_start(tri_sb[:], trimask)
            nc.vector.memset(shift_sb[:], EXP_SHIFT)
            nc.vector.memset(vones[:], 1.0)

            # ---- Interleaved pipeline: A(tch) then B(jq=tch), D at end ----
            # wqk columns: [q0 q1 | k0 k1 | q2 k2]; psum rows 64:128 of each
            # m-tile land on the "wrong" partitions for their head and get
            # staged + partition-shift-DMA'd into place.
            with (
                tc.tile_pool(name="aw", bufs=1) as aw,
                tc.tile_pool(name="ax", bufs=2) as ax,
                tc.tile_pool(name="ast", bufs=3) as ast,
                tc.tile_pool(name="dz", bufs=3) as dz,
                tc.tile_pool(name="bexp", bufs=5) as bexp,
                tc.tile_pool(name="bst", bufs=6) as bst,
                tc.tile_pool(name="bpsS", bufs=3, space="PSUM") as bpsS,
                tc.tile_pool(name="bpsY", bufs=2, space="PSUM") as bpsY,
            ):
                wqk_sb = aw.tile([128, NCCH, 384], F16, tag="wqk")
                wv_sb = aw.tile([128, NCCH, 192], F16, tag="wv")
                wp0_sb = aw.tile([128, C], F16, tag="wp0")
                wp1_sb = aw.tile([64, C], F16, tag="wp1")
                nc.sync.dma_start(wqk_sb[:], wqk.rearrange("(ko p) m -> p ko m", p=128))
                nc.sync.dma_start(wv_sb[:], wv.rearrange("(ko p) m -> p ko m", p=128))
                nc.sync.dma_start(wp0_sb[:], wp[0:128, :])
                nc.sync.dma_start(wp1_sb[:], wp[128:192, :])

                def make_A_groups(tch):
                    tcols = slice(tch * QC, (tch + 1) * QC)
                    xslab = ax.tile([128, NCCH, QC], F16, tag="xslab")
                    nc.sync.dma_start(
                        xslab[:], xT[:, tcols].rearrange("(ko p) t -> p ko t", p=128))

                    def mk_qk(mt):
                        def g():
                            ps = bpsS.tile([128, 1024], F32, tag="psS",
                                           name=f"psA{tch}_{mt}")[:, 0:QC]
                            for cch in range(NCCH):
                                nc.tensor.matmul(
                                    ps[:], wqk_sb[:, cch, mt * 128:(mt + 1) * 128],
                                    xslab[:, cch, :],
                                    start=(cch == 0), stop=(cch == NCCH - 1))
                            lo_dst = [qT[0:64, 0, tcols], kT[0:64, 0, tcols],
                                      qT[0:64, 2, tcols]][mt]
                            hi_dst = [qT[0:64, 1, tcols], kT[0:64, 1, tcols],
                                      kT[0:64, 2, tcols]][mt]
                            nc.vector.tensor_scalar_add(lo_dst, ps[0:64, :],
                                                        bqk_sb[0:64, mt:mt + 1])
                            stg = ast.tile([128, QC], F16, tag="astg")
                            nc.vector.tensor_scalar_add(stg[64:128, :], ps[64:128, :],
                                                        bqk_sb[64:128, mt:mt + 1])
                            nc.sync.dma_start(hi_dst, stg[64:128, :])
                        return g

                    def mk_v(sub):
                        def g():
                            psv = bpsS.tile([128, 1024], F32, tag="psS",
                                            name=f"psV{tch}_{sub}")[:, 0:QC]
                            for cch in range(NCCH):
                                nc.tensor.matmul(
                                    psv[:, 0:192],
                                    xslab[:, cch, sub * 128:(sub + 1) * 128],
                                    wv_sb[:, cch, :],
                                    start=(cch == 0), stop=(cch == NCCH - 1))
                            tt = tch * 4 + sub
                            nc.vector.tensor_copy(vones[:, tt, 0:64], psv[:, 0:64])
                            nc.vector.tensor_copy(vones[:, tt, 128:256],
                                                  psv[:, 64:192])
                        return g

                    return [mk_qk(mt) for mt in range(3)] + [mk_v(s) for s in range(4)]

                def make_proj(tt):
                    def g():
                        tsl = slice(tt * 128, (tt + 1) * 128)
                        pz = bpsS.tile([128, 1024], F32, tag="psS", name=f"pz{tt}")
                        nc.tensor.matmul(pz[:, 0:512], yT0[:, tsl], wp0_sb[:, 0:512],
                                         start=True, stop=False)
                        nc.tensor.matmul(pz[:, 512:768], yT0[:, tsl],
                                         wp0_sb[:, 512:768], start=True, stop=False)
                        nc.tensor.matmul(pz[:, 0:512], yT1[:, tsl], wp1_sb[:, 0:512],
                                         start=False, stop=True)
                        nc.tensor.matmul(pz[:, 512:768], yT1[:, tsl],
                                         wp1_sb[:, 512:768], start=False, stop=True)
                        zt = dz.tile([128, C], F32, tag="zt")
                        nc.vector.tensor_copy(zt[:], pz[:, 0:C])
                        nc.sync.dma_start(z[tsl, :], zt[:])
                    return g

                def emit_B(jq, inserts):
                    total_units = 3 * (jq + 2)
                    spacing = max(1, total_units // max(len(inserts), 1))
                    gu = [0]
                    for h in range(HPC):
                        kTh = kT[0:64, h, :]
                        qTh = qT[0:64, h, :]
                        # units of two kt blocks; the last two units are the
                        # diagonal straddles with shrinking widths.
                        units = [("full", (2 * p, 2 * p + 1)) for p in range(2 * jq)]
                        units += [("diag", (4 * jq, 4 * jq + 1)),
                                  ("diag", (4 * jq + 2, 4 * jq + 3))]
                        nu = len(units)
                        es_info = [None] * nu
                        psY = bpsY.tile([128, QC], F32, tag="psY")

                        def emit_S(ui):
                            kind, kts = units[ui]
                            ps = bpsS.tile([128, 1024], F32, tag="psS")
                            es = bexp.tile([128, 1024], F16, tag="es")
                            offs = []
                            pos = 0
                            for kt in kts:
                                r = kt - 4 * jq
                                off = max(r, 0) * KTS
                                w = QC - off
                                nc.tensor.matmul(
                                    ps[:, pos:pos + w],
                                    kTh[:, kt * KTS:(kt + 1) * KTS],
                                    qTh[:, jq * QC + off:(jq + 1) * QC],
                                    start=True, stop=True)
                                offs.append((kt, off, w, pos))
                                pos += w
                            nc.scalar.activation(es[:, 0:pos], ps[:, 0:pos], AF.Exp,
                                                 bias=shift_sb[:, 0:1])
                            if kind == "diag":
                                for kt, off, w, pos_ in offs:
                                    nc.vector.tensor_mul(
                                        es[:, pos_:pos_ + 128],
                                        es[:, pos_:pos_ + 128], tri_sb[:])
                            es_info[ui] = (es, offs)

                        def emit_AV(ui):
                            es, offs = es_info[ui]
                            vc = V_LHST[h]
                            for j, (kt, off, w, pos_) in enumerate(offs):
                                nc.tensor.matmul(
                                    psY[:, off:QC],
                                    vones[:, kt, vc:vc + 128],
                                    es[:, pos_:pos_ + w],
                                    start=(ui == 0 and j == 0),
                                    stop=(ui == nu - 1 and j == len(offs) - 1))
                            es_info[ui] = None

                        # software pipeline: S-runs and AV-runs batched 2 units
                        emit_S(0)
                        if nu > 1:
                            emit_S(1)
                        for ui in range(nu):
                            if ui % 2 == 0:
                                if ui + 2 < nu:
                                    emit_S(ui + 2)
                                if ui + 3 < nu:
                                    emit_S(ui + 3)
                            emit_AV(ui)
                            gu[0] += 1
                            if inserts and gu[0] % spacing == 0:
                                inserts.pop(0)()

                        # normalize: stage psY out fast (copies free the bank
                        # in ~1.3us so the next rounds' AV can start), then
                        # reciprocal + partition-shift DMA + multiply.
                        ystage = bst.tile([128, QC], F32, tag="ystage")
                        rstage = bst.tile([128, QC], F32, tag="rstage")
                        rt = bst.tile([128, QC], F32, tag="rt")
                        ycols = slice(jq * QC, (jq + 1) * QC)
                        if h == 1:  # [1|V]: rowsum on 0:64, y on 64:128
                            nc.vector.tensor_copy(ystage[64:128, :], psY[64:128, :])
                            nc.vector.reciprocal(rstage[0:64, :], psY[0:64, :])
                            nc.sync.dma_start(rt[64:128, :], rstage[0:64, :])
                            nc.vector.tensor_mul(yT0[64:128, ycols],
                                                 ystage[64:128, :], rt[64:128, :])
                        else:       # [V|1]: y on 0:64, rowsum on 64:128
                            nc.vector.tensor_copy(ystage[0:64, :], psY[0:64, :])
                            nc.vector.reciprocal(rstage[64:128, :], psY[64:128, :])
                            nc.sync.dma_start(rt[0:64, :], rstage[64:128, :])
                            dst = yT0[0:64, ycols] if h == 0 else yT1[:, ycols]
                            nc.vector.tensor_mul(dst, ystage[0:64, :], rt[0:64, :])

                for g in make_A_groups(0):
                    g()
                for jq in range(NJQ):
                    inserts = []
                    if jq + 1 < NJQ:
                        inserts += make_A_groups(jq + 1)
                    if jq >= 1:
                        inserts += [make_proj(tt) for tt in
                                    range((jq - 1) * 4, jq * 4)]
                    emit_B(jq, inserts)
                    for g in inserts:
                        g()

                if debug:
                    nc.sync.dma_start(dq, qT[:])
                    nc.sync.dma_start(dk, kT[:])
                    nc.sync.dma_start(dv, vones[:])
                    nc.sync.dma_start(dy0, yT0[:])
                    nc.sync.dma_start(dy1, yT1[:])

                # remaining projection tiles (last jq round)
                for tt in range((NJQ - 1) * 4, NTT):
                    make_proj(tt)()

    nc.compile()
    return nc


def _get_program():
    if "nc" not in _cache:
        _cache["nc"] = _build()
    return _cache["nc"]


def kernel(x, W_attn, b_attn, W_proj, b_proj):
    global last_results
    from concourse.bass_utils import run_bass_kernel_spmd

    x = np.asarray(x, dtype=np.float32)
    W_attn = np.asarray(W_attn, dtype=np.float32)
    b_attn = np.asarray(b_attn, dtype=np.float32)
    W_proj = np.asarray(W_proj, dtype=np.float32)
    b_proj = np.asarray(b_proj, dtype=np.float32)

    Wq, Wk, Wv = W_attn[:, 0:C], W_attn[:, C:2 * C], W_attn[:, 2 * C:3 * C]
    bq, bk, bv = b_attn[0:C], b_attn[C:2 * C], b_attn[2 * C:3 * C]
    scale = 1.0 / np.sqrt(D)

    xTb = [np.ascontiguousarray(x[b].T).astype(np.float16) for b in range(B)]
    tri = np.triu(np.ones((128, 128), dtype=np.float16))  # keep f >= p

    in_maps = []
    for core in range(NCORE):
        b = core // 4
        h0 = 3 * (core % 4)
        cs = slice(h0 * D, (h0 + HPC) * D)  # this core's 192 channels
        q_w = Wq[:, cs] * scale
        k_w = Wk[:, cs]
        # columns: [q0 q1 | k0 k1 | q2 k2]
        wqk_i = np.concatenate(
            [q_w[:, 0:128], k_w[:, 0:128], q_w[:, 128:192], k_w[:, 128:192]],
            axis=1)
        bq_c = bq[cs] * scale
        bk_c = bk[cs]
        bqk_i = np.zeros((128, 3), dtype=np.float32)
        bqk_i[:, 0] = bq_c[0:128]
        bqk_i[:, 1] = bk_c[0:128]
        bqk_i[0:64, 2] = bq_c[128:192]
        bqk_i[64:128, 2] = bk_c[128:192]
        in_maps.append({
            "xT": xTb[b],
            "wqk": wqk_i.astype(np.float16),
            "wv": np.ascontiguousarray(Wv[:, cs]).astype(np.float16),
            "wp": np.ascontiguousarray(W_proj[cs, :]).astype(np.float16),
            "bqk": bqk_i,
            "trimask": tri,
        })

    nc = _get_program()
    trace = os.environ.get("CC_ATTN_TRACE", "0") == "1"
    res = run_bass_kernel_spmd(nc, in_maps, core_ids=list(range(NCORE)),
                               trace=trace)
    last_results = res

    bias_row = (b_proj + bv @ W_proj).astype(np.float32)  # [768]
    out = np.empty((B, T, C), dtype=np.float32)
    for b in range(B):
        acc = res.results[4 * b]["z"].astype(np.float32).copy()
        for g in range(1, 4):
            acc += res.results[4 * b + g]["z"]
        out[b] = acc + bias_row
    return out


# revision 20
# speedup vs baseline: 1.0391x; 1.0391x over previous
"""Causal self-attention (B=2, T=4096, C=768, H=12, D=64) on 8 Trainium2 cores.

Sharding: (batch, head-group) across 8 cores — core i handles batch i//4,
heads 3*(i%4) .. 3*(i%4)+2.  Each core computes q/k in a transposed [d, T]
layout (S^T formulation: no transposes anywhere in attention), v in natural
[T, d] layout packed next to a ones-block so one AV matmul produces both
y_un^T and the broadcast softmax denominator.  Output projection produces a
partial z[T, C] per core; host sums the 4 partials per batch and adds biases.

Numerics: all matmuls in fp16 (same 10-bit mantissa as TF32/fp32r, but full
PE rate), fp32 PSUM accumulation.  Softmax exp has no max-subtraction; a
constant exp(S-10) shift keeps probs inside fp16 range and cancels in the
normalization.  v-bias and output bias fold into one host-side row:
y @ W_p + b_p == (y0/rowsum) @ W_p + (b_v @ W_p + b_p).

Perf notes (measured on HW): fp16/bf16 matmul N=512 is ~222 ns warm; matmuls
that alternate partition base or contraction row-groups pay ~100 ns per
transition, so all heads keep base-0 operands and S-matmul (K=64) / AV-matmul
(K=128) runs are batched ~4-long.  DVE reciprocal is ~3.4 us per call, so the
normalization chain stages everything to SBUF to free PSUM immediately.
"""
import os
import sys

sys.path.insert(0, "/opt/trn_rl_repo")

import numpy as np

B, T, C = 2, 4096, 768
H, D = 12, 64
HPC = 3            # heads per core
NCORE = 8
QC = 512           # q-chunk (free dim of S^T blocks)
KTS = 128          # k-tile size
NJQ = T // QC      # 8 q-chunks
NKT = T // KTS     # 32 k-tiles
NTT = T // 128     # 32 t-tiles (proj)
NCCH = C // 128    # 6 contraction chunks

# vones column layout: [v0 | ones | v1 | v2 | ones]
VONES_W = 320
V_LHST = [0, 64, 192]    # lhsT col offset per local head ([V|1], [1|V], [V|1])
V_DST = [0, 128, 192]    # where phase A writes each head's v block
EXP_SHIFT = -10.0

_cache = {}
last_results = None  # set by kernel(); test.py reads exec_time_ns off this


def _build():
    import concourse.mybir as mybir
    import concourse.tile as tile
    from concourse import bacc

    F32 = mybir.dt.float32
    F16 = mybir.dt.float16
    AF = mybir.ActivationFunctionType

    nc = bacc.Bacc("TRN2", target_bir_lowering=False, debug=False)

    xT = nc.dram_tensor("xT", [C, T], F16, kind="ExternalInput").ap()
    wqk = nc.dram_tensor("wqk", [C, 384], F16, kind="ExternalInput").ap()
    wv = nc.dram_tensor("wv", [C, 192], F16, kind="ExternalInput").ap()
    wp = nc.dram_tensor("wp", [192, C], F16, kind="ExternalInput").ap()
    bqk = nc.dram_tensor("bqk", [128, 3], F32, kind="ExternalInput").ap()
    trimask = nc.dram_tensor("trimask", [128, 128], F16, kind="ExternalInput").ap()
    z = nc.dram_tensor("z", [T, C], F32, kind="ExternalOutput").ap()
    debug = os.environ.get("CC_ATTN_DEBUG", "0") == "1"
    if debug:
        dq = nc.dram_tensor("dbg_qT", [64, HPC, T], F16, kind="ExternalOutput").ap()
        dk = nc.dram_tensor("dbg_kT", [64, HPC, T], F16, kind="ExternalOutput").ap()
        dv = nc.dram_tensor("dbg_vones", [128, 32 * VONES_W], F16,
                            kind="ExternalOutput").ap()
        dy0 = nc.dram_tensor("dbg_yT0", [128, T], F16, kind="ExternalOutput").ap()
        dy1 = nc.dram_tensor("dbg_yT1", [64, T], F16, kind="ExternalOutput").ap()

    with tile.TileContext(nc) as tc:
        with tc.tile_pool(name="persist", bufs=1) as persist:
            qT = persist.tile([64, HPC, T], F16, tag="qT")
            kT = persist.tile([64, HPC, T], F16, tag="kT")
            vones = persist.tile([128, NKT, VONES_W], F16, tag="vones")
            yT0 = persist.tile([128, T], F16, tag="yT0")
            yT1 = persist.tile([64, T], F16, tag="yT1")
            bqk_sb = persist.tile([128, 3], F32, tag="bqk")
            shift_sb = persist.tile([128, 1], F32, tag="shift")
            tri_sb = persist.tile([128, 128], F16, tag="tri")

            nc.sync.dma_start(bqk_sb[:], bqk)
            nc.sync.dma_start(tri_sb[:], trimask)
            nc.vector.memset(shift_sb[:], EXP_SHIFT)
            nc.vector.memset(vones[:], 1.0)

            # ---- Interleaved pipeline: A(tch) then B(jq=tch), D at end ----
            # wqk columns: [q0 q1 | k0 k1 | q2 k2]; psum rows 64:128 of each
            # m-tile land on the "wrong" partitions for their head and get
            # staged + partition-shift-DMA'd into place.
            with (
                tc.tile_pool(name="aw", bufs=1) as aw,
                tc.tile_pool(name="ax", bufs=2) as ax,
                tc.tile_pool(name="ast", bufs=3) as ast,
                tc.tile_pool(name="dz", bufs=3) as dz,
                tc.tile_pool(name="bexp", bufs=5) as bexp,
                tc.tile_pool(name="bst", bufs=6) as bst,
                tc.tile_pool(name="bpsS", bufs=3, space="PSUM") as bpsS,
                tc.tile_pool(name="bpsY", bufs=2, space="PSUM") as bpsY,
            ):
                wqk_sb = aw.tile([128, NCCH, 384], F16, tag="wqk")
                wv_sb = aw.tile([128, NCCH, 192], F16, tag="wv")
                wp0_sb = aw.tile([128, C], F16, tag="wp0")
                wp1_sb = aw.tile([64, C], F16, tag="wp1")
                nc.sync.dma_start(wqk_sb[:], wqk.rearrange("(ko p) m -> p ko m", p=128))
                nc.sync.dma_start(wv_sb[:], wv.rearrange("(ko p) m -> p ko m", p=128))
                nc.sync.dma_start(wp0_sb[:], wp[0:128, :])
                nc.sync.dma_start(wp1_sb[:], wp[128:192, :])

                def make_A_groups(tch):
                    tcols = slice(tch * QC, (tch + 1) * QC)
                    xslab = ax.tile([128, NCCH, QC], F16, tag="xslab")
                    nc.sync.dma_start(
                        xslab[:], xT[:, tcols].rearrange("(ko p) t -> p ko t", p=128))

                    def mk_qk(mt):
                        def g():
                            ps = bpsS.tile([128, 1024], F32, tag="psS",
                                           name=f"psA{tch}_{mt}")[:, 0:QC]
                            for cch in range(NCCH):
                                nc.tensor.matmul(
                                    ps[:], wqk_sb[:, cch, mt * 128:(mt + 1) * 128],
                                    xslab[:, cch, :],
                                    start=(cch == 0), stop=(cch == NCCH - 1))
                            lo_dst = [qT[0:64, 0, tcols], kT[0:64, 0, tcols],
                                      qT[0:64, 2, tcols]][mt]
                            hi_dst = [qT[0:64, 1, tcols], kT[0:64, 1, tcols],
                                      kT[0:64, 2, tcols]][mt]
                            nc.vector.tensor_scalar_add(lo_dst, ps[0:64, :],
                                                        bqk_sb[0:64, mt:mt + 1])
                            stg = ast.tile([128, QC], F16, tag="astg")
                            nc.vector.tensor_scalar_add(stg[64:128, :], ps[64:128, :],
                                                        bqk_sb[64:128, mt:mt + 1])
                            nc.sync.dma_start(hi_dst, stg[64:128, :])
                        return g

                    def mk_v(sub):
                        def g():
                            psv = bpsS.tile([128, 1024], F32, tag="psS",
                                            name=f"psV{tch}_{sub}")[:, 0:QC]
                            for cch in range(NCCH):
                                nc.tensor.matmul(
                                    psv[:, 0:192],
                                    xslab[:, cch, sub * 128:(sub + 1) * 128],
                                    wv_sb[:, cch, :],
                                    start=(cch == 0), stop=(cch == NCCH - 1))
                            tt = tch * 4 + sub
                            nc.vector.tensor_copy(vones[:, tt, 0:64], psv[:, 0:64])
                            nc.vector.tensor_copy(vones[:, tt, 128:256],
                                                  psv[:, 64:192])
                        return g

                    return [mk_qk(mt) for mt in range(3)] + [mk_v(s) for s in range(4)]

                def make_proj(tt):
                    def g():
                        tsl = slice(tt * 128, (tt + 1) * 128)
                        pz = bpsS.tile([128, 1024], F32, tag="psS", name=f"pz{tt}")
                        nc.tensor.matmul(pz[:, 0:512], yT0[:, tsl], wp0_sb[:, 0:512],
                                         start=True, stop=False)
                        nc.tensor.matmul(pz[:, 512:768], yT0[:, tsl],
                                         wp0_sb[:, 512:768], start=True, stop=False)
                        nc.tensor.matmul(pz[:, 0:512], yT1[:, tsl], wp1_sb[:, 0:512],
                                         start=False, stop=True)
                        nc.tensor.matmul(pz[:, 512:768], yT1[:, tsl],
                                         wp1_sb[:, 512:768], start=False, stop=True)
                        zt = dz.tile([128, C], F32, tag="zt")
                        nc.vector.tensor_copy(zt[:], pz[:, 0:C])
                        nc.sync.dma_start(z[tsl, :], zt[:])
                    return g

                def emit_B(jq, inserts):
                    total_units = 3 * (jq + 2)
                    spacing = max(1, total_units // max(len(inserts), 1))
                    gu = [0]
                    for h in range(HPC):
                        kTh = kT[0:64, h, :]
                        qTh = qT[0:64, h, :]
                        # units of two kt blocks; the last two units are the
                        # diagonal straddles with shrinking widths.
                        units = [("full", (2 * p, 2 * p + 1)) for p in range(2 * jq)]
                        units += [("diag", (4 * jq, 4 * jq + 1)),
                                  ("diag", (4 * jq + 2, 4 * jq + 3))]
                        nu = len(units)
                        es_info = [None] * nu
                        psY = bpsY.tile([128, QC], F32, tag="psY")

                        def emit_S(ui):
                            kind, kts = units[ui]
                            ps = bpsS.tile([128, 1024], F32, tag="psS")
                            es = bexp.tile([128, 1024], F16, tag="es")
                            offs = []
                            pos = 0
                            for kt in kts:
                                r = kt - 4 * jq
                                off = max(r, 0) * KTS
                                w = QC - off
                                nc.tensor.matmul(
                                    ps[:, pos:pos + w],
                                    kTh[:, kt * KTS:(kt + 1) * KTS],
                                    qTh[:, jq * QC + off:(jq + 1) * QC],
                                    start=True, stop=True)
                                offs.append((kt, off, w, pos))
                                pos += w
                            nc.scalar.activation(es[:, 0:pos], ps[:, 0:pos], AF.Exp,
                                                 bias=shift_sb[:, 0:1])
                            if kind == "diag":
                                for kt, off, w, pos_ in offs:
                                    nc.vector.tensor_mul(
                                        es[:, pos_:pos_ + 128],
                                        es[:, pos_:pos_ + 128], tri_sb[:])
                            es_info[ui] = (es, offs)

                        def emit_AV(ui):
                            es, offs = es_info[ui]
                            vc = V_LHST[h]
                            for j, (kt, off, w, pos_) in enumerate(offs):
                                nc.tensor.matmul(
                                    psY[:, off:QC],
                                    vones[:, kt, vc:vc + 128],
                                    es[:, pos_:pos_ + w],
                                    start=(ui == 0 and j == 0),
                                    stop=(ui == nu - 1 and j == len(offs) - 1))
                            es_info[ui] = None

                        # software pipeline: S-runs and AV-runs batched 2 units
                        emit_S(0)
                        if nu > 1:
                            emit_S(1)
                        for ui in range(nu):
                            if ui % 2 == 0:
                                if ui + 2 < nu:
                                    emit_S(ui + 2)
                                if ui + 3 < nu:
                                    emit_S(ui + 3)
                            emit_AV(ui)
                            gu[0] += 1
                            if inserts and gu[0] % spacing == 0:
                                inserts.pop(0)()

                        # normalize: stage psY out fast (copies free the bank
                        # in ~1.3us so the next rounds' AV can start), then
                        # reciprocal + partition-shift DMA + multiply.
                        ystage = bst.tile([128, QC], F32, tag="ystage")
                        rstage = bst.tile([128, QC], F32, tag="rstage")
                        rt = bst.tile([128, QC], F32, tag="rt")
                        ycols = slice(jq * QC, (jq + 1) * QC)
                        if h == 1:  # [1|V]: rowsum on 0:64, y on 64:128
                            nc.vector.tensor_copy(ystage[64:128, :], psY[64:128, :])
                            nc.vector.tensor_copy(rstage[0:64, :], psY[0:64, :])
                            nc.vector.reciprocal(rstage[0:64, :], rstage[0:64, :])
                            nc.sync.dma_start(rt[64:128, :], rstage[0:64, :])
                            nc.vector.tensor_mul(yT0[64:128, ycols],
                                                 ystage[64:128, :], rt[64:128, :])
                        else:       # [V|1]: y on 0:64, rowsum on 64:128
                            nc.vector.tensor_copy(ystage[0:64, :], psY[0:64, :])
                            nc.vector.tensor_copy(rstage[64:128, :], psY[64:128, :])
                            nc.vector.reciprocal(rstage[64:128, :], rstage[64:128, :])
                            nc.sync.dma_start(rt[0:64, :], rstage[64:128, :])
                            dst = yT0[0:64, ycols] if h == 0 else yT1[:, ycols]
                            nc.vector.tensor_mul(dst, ystage[0:64, :], rt[0:64, :])

                for g in make_A_groups(0):
                    g()
                for jq in range(NJQ):
                    inserts = []
                    if jq + 1 < NJQ:
                        inserts += make_A_groups(jq + 1)
                    if jq >= 1:
                        inserts += [make_proj(tt) for tt in
                                    range((jq - 1) * 4, jq * 4)]
                    emit_B(jq, inserts)
                    for g in inserts:
                        g()

                if debug:
                    nc.sync.dma_start(dq, qT[:])
                    nc.sync.dma_start(dk, kT[:])
                    nc.sync.dma_start(dv, vones[:])
                    nc.sync.dma_start(dy0, yT0[:])
                    nc.sync.dma_start(dy1, yT1[:])

                # remaining projection tiles (last jq round)
                for tt in range((NJQ - 1) * 4, NTT):
                    make_proj(tt)()

    nc.compile()
    return nc


def _get_program():
    if "nc" not in _cache:
        _cache["nc"] = _build()
    return _cache["nc"]


def kernel(x, W_attn, b_attn, W_proj, b_proj):
    global last_results
    from concourse.bass_utils import run_bass_kernel_spmd

    x = np.asarray(x, dtype=np.float32)
    W_attn = np.asarray(W_attn, dtype=np.float32)
    b_attn = np.asarray(b_attn, dtype=np.float32)
    W_proj = np.asarray(W_proj, dtype=np.float32)
    b_proj = np.asarray(b_proj, dtype=np.float32)

    Wq, Wk, Wv = W_attn[:, 0:C], W_attn[:, C:2 * C], W_attn[:, 2 * C:3 * C]
    bq, bk, bv = b_attn[0:C], b_attn[C:2 * C], b_attn[2 * C:3 * C]
    scale = 1.0 / np.sqrt(D)

    xTb = [np.ascontiguousarray(x[b].T).astype(np.float16) for b in range(B)]
    tri = np.triu(np.ones((128, 128), dtype=np.float16))  # keep f >= p

    in_maps = []
    for core in range(NCORE):
        b = core // 4
        h0 = 3 * (core % 4)
        cs = slice(h0 * D, (h0 + HPC) * D)  # this core's 192 channels
        q_w = Wq[:, cs] * scale
        k_w = Wk[:, cs]
        # columns: [q0 q1 | k0 k1 | q2 k2]
        wqk_i = np.concatenate(
            [q_w[:, 0:128], k_w[:, 0:128], q_w[:, 128:192], k_w[:, 128:192]],
            axis=1)
        bq_c = bq[cs] * scale
        bk_c = bk[cs]
        bqk_i = np.zeros((128, 3), dtype=np.float32)
        bqk_i[:, 0] = bq_c[0:128]
        bqk_i[:, 1] = bk_c[0:128]
        bqk_i[0:64, 2] = bq_c[128:192]
        bqk_i[64:128, 2] = bk_c[128:192]
        in_maps.append({
            "xT": xTb[b],
            "wqk": wqk_i.astype(np.float16),
            "wv": np.ascontiguousarray(Wv[:, cs]).astype(np.float16),
            "wp": np.ascontiguousarray(W_proj[cs, :]).astype(np.float16),
            "bqk": bqk_i,
            "trimask": tri,
        })

    nc = _get_program()
    trace = os.environ.get("CC_ATTN_TRACE", "0") == "1"
    res = run_bass_kernel_spmd(nc, in_maps, core_ids=list(range(NCORE)),
                               trace=trace)
    last_results = res

    bias_row = (b_proj + bv @ W_proj).astype(np.float32)  # [768]
    out = np.empty((B, T, C), dtype=np.float32)
    for b in range(B):
        acc = res.results[4 * b]["z"].astype(np.float32).copy()
        for g in range(1, 4):
            acc += res.results[4 * b + g]["z"]
        out[b] = acc + bias_row
    return out
